# revision 3
# baseline (speedup 1.0000x reference)
"""GPT-mini forward on 8 NeuronCores (Trainium2, Bass/Tile), v3.

Sharding: core c = 2b+s handles tokens [s*512,(s+1)*512) of batch b=c//2
(sequence split, no redundant trunk GEMMs). Per layer, the LN1 output h
is AllGathered within the pair (2-rank groups) as fp8e4m3 (0.5MB), issued
immediately after LN1; each core then recomputes the rank-even peer's K,V
from the gathered h while the local K/V/Q/attention fills the comm window.
Remote-key attention is disabled on s=0 cores via an exp bias of -100
(their queries precede all remote keys). Head: full vocab x local tokens.

Precision: residual x + LN in fp32(r); weight GEMMs and attention bf16
(weights shipped bf16); fp8 only on the wire for the peer-h payload;
fp32 PSUM accumulation everywhere; logits returned via bf16 DMA.
"""

import sys

sys.path.insert(0, "/opt/trn_rl_repo")

import numpy as np

import concourse.bass as bass
import concourse.bacc as bacc
import concourse.mybir as mybir
from concourse import tile
from concourse.bass_utils import run_bass_kernel_spmd

V, BLOCK, D, L, H, B, T = 32000, 1024, 1024, 6, 16, 4, 1024
HD = D // H          # 64
FF = 4 * D           # 4096
NCORES = 8
P = 128
TL = 512             # tokens per core (sequence split)
CT = D // P          # 8 c-tiles
FT = FF // P         # 32 ff-tiles
KTL = TL // P        # 4 local key tiles
VT = V // P          # 250 head tiles
OBAT = 8             # head out-tiles per output DMA
VXW = H * (HD + 1)   # 1040 (V-ext width incl ones cols)
NH = (D // P) * P * TL  # gathered-h payload elems (fp8)

F32 = mybir.dt.float32
F32R = mybir.dt.float32r
BF16 = mybir.dt.bfloat16
FP8 = mybir.dt.float8e4
AF = mybir.ActivationFunctionType
OP = mybir.AluOpType

# packed small-param layout (columns of one [P, PKW] f32 input)
LAB_O = 0                      # ln1 g/b interleaved + lnf g
LAB2_O = LAB_O + 2 * CT * L + CT
BB1_O = LAB2_O + 2 * CT * L + CT
BB2_O = BB1_O + FT * L
PKW = BB2_O + CT * L


def r(ap):
    return ap.bitcast(F32R)


def build_program():
    nc = bacc.Bacc("TRN2", target_bir_lowering=False, debug=False,
                   num_devices=NCORES)

    # ---- I/O ----
    x0T = nc.declare_dram_parameter("x0T", [D, TL], F32R, isOutput=False)
    qkvw = nc.declare_dram_parameter("qkvw", [L, D, 3 * D], BF16, isOutput=False)
    projw = nc.declare_dram_parameter("projw", [L, D, D], BF16, isOutput=False)
    w1 = nc.declare_dram_parameter("w1", [L, D, FF], BF16, isOutput=False)
    w2 = nc.declare_dram_parameter("w2", [L, FF, D], BF16, isOutput=False)
    headw = nc.declare_dram_parameter("headw", [D, V], BF16, isOutput=False)
    packed = nc.declare_dram_parameter("packed", [P, PKW], F32, isOutput=False)
    masks = nc.declare_dram_parameter("masks", [4, P, TL], F32, isOutput=False)
    rbias = nc.declare_dram_parameter("rbias", [P, 1], F32, isOutput=False)
    onesd = nc.declare_dram_parameter("onesd", [P, 1], F32R, isOutput=False)
    onesrd = nc.declare_dram_parameter("onesrd", [1, P], F32R, isOutput=False)
    identd = nc.declare_dram_parameter("identd", [P, P], F32, isOutput=False)
    out = nc.declare_dram_parameter("out", [V, TL], BF16, isOutput=True)

    with tile.TileContext(nc) as tc:
        with (
            nc.allow_low_precision(reason="bf16 weights/attention pipeline"),
            tc.tile_pool(name="persist", bufs=1) as pp,
            tc.tile_pool(name="hy", bufs=1) as hyp,
            tc.tile_pool(name="h8p", bufs=1) as h8p,
            tc.tile_pool(name="qp", bufs=1) as qp,
            tc.tile_pool(name="kv", bufs=1) as kvp,
            tc.tile_pool(name="big", bufs=1) as bigp,
            tc.tile_pool(name="ob", bufs=2) as obp,
            tc.tile_pool(name="wq", bufs=7) as wqp,
            tc.tile_pool(name="w2p", bufs=2) as w2p,
            tc.tile_pool(name="vt", bufs=3) as vtp,
            tc.tile_pool(name="ep", bufs=8) as ep,
            tc.tile_pool(name="sm", bufs=4) as smp,
            tc.tile_pool(name="psA", bufs=3, space="PSUM") as psA,
            tc.tile_pool(name="psB", bufs=3, space="PSUM") as psB,
            tc.tile_pool(name="dram", bufs=2, space="DRAM") as dramp,
        ):
            # ---- persistent SBUF state ----
            xt = [pp.tile([P, TL], F32R, tag=f"xt{i}", name=f"xt{i}") for i in range(CT)]
            maskt = pp.tile([P, 4, TL], F32, tag="masks", name="maskt")
            maskb = pp.tile([P, 4, TL], BF16, tag="masksb", name="maskb")
            ones = pp.tile([P, 1], F32R, tag="ones", name="ones")
            onesr = pp.tile([1, P], F32R, tag="onesr", name="onesr")
            rbias_t = pp.tile([P, 1], F32, tag="rbias", name="rbias_t")
            idf = pp.tile([P, P], F32, tag="idf", name="idf")
            identb = pp.tile([P, P], BF16, tag="identb", name="identb")
            pk = pp.tile([P, PKW], F32, tag="pk", name="pk")

            nc.sync.dma_start(maskt[:], masks.ap().rearrange("a p f -> p a f"))
            nc.sync.dma_start(ones[:], onesd[:, :])
            nc.sync.dma_start(onesr[:], onesrd[:, :])
            nc.sync.dma_start(rbias_t[:], rbias[:, :])
            nc.sync.dma_start(idf[:], identd[:, :])
            nc.sync.dma_start(pk[:], packed[:, :])
            nc.vector.tensor_copy(identb[:], idf[:])
            for mi in range(4):
                nc.vector.tensor_copy(maskb[:, mi, :], maskt[:, mi, :])
            for i in range(CT):
                nc.sync.dma_start(xt[i][:], x0T[i * P:(i + 1) * P, :])
            lnf_off = L * 2 * CT

            def layernorm(src_tiles, gb_off, dst, final=False, h8_tile=None):
                """dst = LN(src), dst is one [P, CT, TL] tile. Normal layers:
                g/b interleaved at pk[gb_off]. final=True: g at LAB_O+lnf_off+k,
                b at LAB2_O+lnf_off+k. h8_tile: extra fp8 copy of dst."""
                s_ps = psB.tile([1, TL], F32, tag="st0", name="st0", bufs=1)[:]
                q_ps = psB.tile([1, TL], F32, tag="st1", name="st1", bufs=1)[:]
                sqs = []
                for k in range(CT):
                    sq = smp.tile([P, TL], F32R, tag="scr", name="sq", bufs=2)
                    nc.vector.tensor_mul(sq[:], src_tiles[k][:], src_tiles[k][:])
                    sqs.append(sq)
                for k in range(CT):
                    nc.tensor.matmul(s_ps, r(ones[:]), r(src_tiles[k][:]),
                                     start=(k == 0), stop=(k == CT - 1))
                for k in range(CT):
                    nc.tensor.matmul(q_ps, r(ones[:]), r(sqs[k][:]),
                                     start=(k == 0), stop=(k == CT - 1))
                mu = smp.tile([1, TL], F32R, tag="st", name="mu", bufs=1)
                rstd = smp.tile([1, TL], F32R, tag="st2", name="rstd", bufs=1)
                nc.vector.tensor_scalar_mul(mu[:], s_ps, 1.0 / D)
                nc.vector.tensor_mul(rstd[:], mu[:], mu[:])
                nc.vector.scalar_tensor_tensor(rstd[:], q_ps, 1.0 / D, rstd[:],
                                               OP.mult, OP.subtract)
                nc.vector.tensor_scalar_add(rstd[:], rstd[:], 1e-5)
                nc.scalar.activation(rstd[:], rstd[:], AF.Sqrt)
                nc.vector.reciprocal(rstd[:], rstd[:])
                mu_bc = psA.tile([P, TL], F32, tag="a", name="mubc")
                rs_bc = psA.tile([P, TL], F32, tag="a", name="rsbc")
                nc.tensor.matmul(mu_bc[:], r(onesr[:]), r(mu[:]),
                                 start=True, stop=True)
                nc.tensor.matmul(rs_bc[:], r(onesr[:]), r(rstd[:]),
                                 start=True, stop=True)
                for k in range(CT):
                    tmp = smp.tile([P, TL], F32, tag="scr", name="nrm", bufs=2)
                    nc.vector.tensor_sub(tmp[:], src_tiles[k][:], mu_bc[:])
                    nc.vector.tensor_mul(tmp[:], tmp[:], rs_bc[:])
                    if final:
                        g = pk[:, LAB_O + lnf_off + k:LAB_O + lnf_off + k + 1]
                        bcol = pk[:, LAB2_O + lnf_off + k:LAB2_O + lnf_off + k + 1]
                    else:
                        g = pk[:, gb_off + 2 * k:gb_off + 2 * k + 1]
                        bcol = pk[:, gb_off + 2 * k + 1:gb_off + 2 * k + 2]
                    nc.vector.tensor_scalar(dst[:, k, :], tmp[:], g, bcol,
                                            OP.mult, OP.add)
                if h8_tile is not None:
                    nc.vector.tensor_copy(h8_tile[:], dst[:])

            def compute_kv(h_src, kt_dst, vx_dst, li):
                """K,V GEMMs from h_src ([P,CT,TL] bf16) into kt_dst (K^T
                bf16 tiles) and vx_dst ([P,KTL,VXW] key-major V-ext)."""
                nc.vector.memset(vx_dst[:], 1.0)
                for f in range(CT):
                    wt = wqp.tile([P, CT, P], BF16, tag="wq", name="wq")
                    nc.sync.dma_start(
                        wt[:], qkvw[li, :, D + f * P:D + (f + 1) * P]
                        .rearrange("(a p) f -> p a f", p=P))
                    ps = psA.tile([P, TL], F32, tag="a", name="psa")
                    for k in range(CT):
                        nc.tensor.matmul(ps[:], wt[:, k, :], h_src[:, k, :],
                                         start=(k == 0), stop=(k == CT - 1))
                    nc.vector.tensor_copy(kt_dst[f][:], ps[:])
                for f in range(CT):
                    wt = wqp.tile([P, CT, P], BF16, tag="wq", name="wq")
                    nc.sync.dma_start(
                        wt[:], qkvw[li, :, 2 * D + f * P:2 * D + (f + 1) * P]
                        .rearrange("(a p) f -> p a f", p=P))
                    ps = psA.tile([P, TL], F32, tag="a", name="psa")
                    for k in range(CT):
                        nc.tensor.matmul(ps[:], wt[:, k, :], h_src[:, k, :],
                                         start=(k == 0), stop=(k == CT - 1))
                    vtt = vtp.tile([P, TL], BF16, tag="vt", name="vtt")
                    nc.vector.tensor_copy(vtt[:], ps[:])
                    h0 = 2 * f
                    for t in range(KTL):
                        tp = psB.tile([P, P], BF16, tag="b", name="pstr")
                        nc.tensor.transpose(tp[:], vtt[:, t * P:(t + 1) * P], identb[:])
                        nc.vector.tensor_copy(
                            vx_dst[:, t, h0 * (HD + 1):(h0 + 2) * (HD + 1)]
                            .rearrange("p (h e) -> p h e", e=HD + 1)[:, :, 0:HD],
                            tp[:].rearrange("p (h e) -> p h e", e=HD))

            def attention_block(kt_src, vx_src, qy, avl, local):
                """scores+exp(+mask)+av over one 512-key block for all heads.
                local: accumulate av into avl tiles; else combine with avl
                and write normalized output back into qy."""
                for hh in range(H):
                    ft, row = hh // 2, (hh % 2) * HD
                    es = []
                    for ki in range(KTL):
                        ssp = psA.tile([P, TL], F32, tag="a", name="psa")
                        nc.tensor.matmul(
                            ssp[:], kt_src[ft][row:row + HD, ki * P:(ki + 1) * P],
                            qy[ft][row:row + HD, :], start=True, stop=True)
                        e = ep.tile([P, TL], BF16, tag="e", name="e")
                        if local:
                            nc.scalar.activation(e[:], ssp[:], AF.Exp)
                            nc.vector.tensor_mul(e[:], e[:], maskb[:, ki, :])
                        else:
                            nc.scalar.activation(e[:], ssp[:], AF.Exp,
                                                 bias=rbias_t[:, 0:1])
                        es.append(e)
                    av = psB.tile([HD + 1, TL], F32, tag="b", name="psav")
                    for ki in range(KTL):
                        nc.tensor.matmul(
                            av[:], vx_src[:, ki, hh * (HD + 1):(hh + 1) * (HD + 1)],
                            es[ki][:], start=(ki == 0), stop=(ki == KTL - 1))
                    if local:
                        nc.vector.tensor_copy(avl[hh][:], av[:])
                    else:
                        avs = smp.tile([HD + 1, TL], F32, tag="avs", name="avs", bufs=2)
                        nc.vector.tensor_add(avs[:], avl[hh][:], av[:])
                        rec = smp.tile([1, TL], F32R, tag="st", name="rec", bufs=1)
                        nc.vector.reciprocal(rec[:], avs[HD:HD + 1, :])
                        rec_bc = psB.tile([HD, TL], F32, tag="b", name="recbc")
                        nc.tensor.matmul(rec_bc[:], r(onesr[0:1, 0:HD]), r(rec[:]),
                                         start=True, stop=True)
                        nc.vector.tensor_mul(qy[ft][row:row + HD, :],
                                             avs[0:HD, :], rec_bc[:])

            # =================== layers ===================
            for li in range(L):
                h = hyp.tile([P, CT, TL], BF16, tag="hy", name="h")
                h8 = h8p.tile([P, CT, TL], FP8, tag="h8", name="h8")
                layernorm(xt, LAB_O + li * 2 * CT, h, h8_tile=h8)

                hx_in = dramp.tile([NH], FP8, tag="hxin", name="hxin")
                hx_out = dramp.tile([2 * NH], FP8, tag="hxout", name="hxout")
                nc.gpsimd.dma_start(
                    hx_in[:].rearrange("(p a f) -> p a f", p=P, a=CT), h8[:])
                nc.gpsimd.collective_compute(
                    "AllGather", OP.bypass,
                    replica_groups=[[0, 1], [2, 3], [4, 5], [6, 7]],
                    ins=[hx_in[:].opt()], outs=[hx_out[:].opt()])

                # local K,V while the AllGather is in flight
                kt_loc = [kvp.tile([P, TL], BF16, tag=f"kl{i}", name=f"kl{i}")
                          for i in range(CT)]
                vx_loc = kvp.tile([P, KTL, VXW], BF16, tag="vl", name="vl")
                compute_kv(h, kt_loc, vx_loc, li)

                # Q
                qy = [qp.tile([P, TL], BF16, tag=f"q{i}", name=f"qy{i}")
                      for i in range(CT)]
                for f in range(CT):
                    wt = wqp.tile([P, CT, P], BF16, tag="wq", name="wq")
                    nc.sync.dma_start(
                        wt[:], qkvw[li, :, f * P:(f + 1) * P]
                        .rearrange("(a p) f -> p a f", p=P))
                    ps = psA.tile([P, TL], F32, tag="a", name="psa")
                    for k in range(CT):
                        nc.tensor.matmul(ps[:], wt[:, k, :], h[:, k, :],
                                         start=(k == 0), stop=(k == CT - 1))
                    nc.vector.tensor_scalar_mul(qy[f][:], ps[:], 0.125)

                # local attention (diag masks)
                avl = [kvp.tile([HD + 1, TL], F32, tag=f"av{i}", name=f"av{i}")
                       for i in range(H)]
                attention_block(kt_loc, vx_loc, qy, avl, local=True)

                # peer h -> recompute rank-even's K,V
                h8r = h8p.tile([P, CT, TL], FP8, tag="h8", name="h8r")
                hr = hyp.tile([P, CT, TL], BF16, tag="hy", name="hr")
                nc.gpsimd.dma_start(
                    h8r[:], hx_out[0:NH].rearrange("(p a f) -> p a f", p=P, a=CT))
                nc.vector.tensor_copy(hr[:], h8r[:])
                kt_rem = [kvp.tile([P, TL], BF16, tag=f"kr{i}", name=f"kr{i}")
                          for i in range(CT)]
                vx_rem = kvp.tile([P, KTL, VXW], BF16, tag="vr", name="vr")
                compute_kv(hr, kt_rem, vx_rem, li)

                # remote attention (zeroed on s=0 via exp bias) + combine
                attention_block(kt_rem, vx_rem, qy, avl, local=False)

                # proj + residual
                for f in range(CT):
                    wt = wqp.tile([P, CT, P], BF16, tag="wq", name="wq")
                    nc.sync.dma_start(
                        wt[:], projw[li, :, f * P:(f + 1) * P]
                        .rearrange("(a p) f -> p a f", p=P))
                    ps = psA.tile([P, TL], F32, tag="a", name="psa")
                    for k in range(CT):
                        nc.tensor.matmul(ps[:], wt[:, k, :], qy[k][:],
                                         start=(k == 0), stop=(k == CT - 1))
                    nc.vector.tensor_add(xt[f][:], xt[f][:], ps[:])

                # LN2 -> h
                h = hyp.tile([P, CT, TL], BF16, tag="hy", name="h2")
                layernorm(xt, LAB2_O + li * 2 * CT, h)

                # MLP in two FF halves
                for half in range(2):
                    h1 = [bigp.tile([P, TL], BF16, tag=f"big{i}", name=f"h1{i}")
                          for i in range(FT // 2)]
                    for fl in range(FT // 2):
                        f = half * (FT // 2) + fl
                        wt = wqp.tile([P, CT, P], BF16, tag="wq", name="wq")
                        nc.sync.dma_start(
                            wt[:], w1[li, :, f * P:(f + 1) * P]
                            .rearrange("(a p) f -> p a f", p=P))
                        ps = psA.tile([P, TL], F32, tag="a", name="psa")
                        for k in range(CT):
                            nc.tensor.matmul(ps[:], wt[:, k, :], h[:, k, :],
                                             start=(k == 0), stop=(k == CT - 1))
                        nc.scalar.activation(
                            h1[fl][:], ps[:], AF.Gelu,
                            bias=pk[:, BB1_O + li * FT + f:BB1_O + li * FT + f + 1])
                    for dtile in range(CT):
                        wt2a = w2p.tile([P, FT // 4, P], BF16, tag="w2", name="w2a")
                        wt2b = w2p.tile([P, FT // 4, P], BF16, tag="w2", name="w2b")
                        base = half * (FF // 2)
                        nc.sync.dma_start(
                            wt2a[:], w2[li, base:base + FF // 4,
                                        dtile * P:(dtile + 1) * P]
                            .rearrange("(a p) f -> p a f", p=P))
                        nc.sync.dma_start(
                            wt2b[:], w2[li, base + FF // 4:base + FF // 2,
                                        dtile * P:(dtile + 1) * P]
                            .rearrange("(a p) f -> p a f", p=P))
                        ps = psB.tile([P, TL], F32, tag="b", name="psb")
                        for fl in range(FT // 2):
                            wsel = wt2a if fl < FT // 4 else wt2b
                            nc.tensor.matmul(ps[:], wsel[:, fl % (FT // 4), :],
                                             h1[fl][:],
                                             start=(fl == 0), stop=(fl == FT // 2 - 1))
                        nc.vector.tensor_add(xt[dtile][:], xt[dtile][:], ps[:])
                for dtile in range(CT):
                    nc.vector.tensor_scalar_add(
                        xt[dtile][:], xt[dtile][:],
                        pk[:, BB2_O + li * CT + dtile:BB2_O + li * CT + dtile + 1])

            # =================== final LN + head ===================
            h = hyp.tile([P, CT, TL], BF16, tag="hy", name="hf")
            layernorm(xt, 0, h, final=True)

            def head_batch(vstart, n):
                ob = obp.tile([P, OBAT, TL], BF16, tag="ob", name="ob")
                for vo in range(n):
                    v = vstart + vo
                    wt = wqp.tile([P, CT, P], BF16, tag="wq", name="wq")
                    nc.sync.dma_start(
                        wt[:], headw[:, v * P:(v + 1) * P]
                        .rearrange("(a p) f -> p a f", p=P))
                    ps = psA.tile([P, TL], F32, tag="a", name="psa")
                    for k in range(CT):
                        nc.tensor.matmul(ps[:], wt[:, k, :], h[:, k, :],
                                         start=(k == 0), stop=(k == CT - 1))
                    nc.vector.tensor_copy(ob[:, vo, :], ps[:])
                nc.gpsimd.dma_start(
                    out[vstart * P:(vstart + n) * P, :]
                    .rearrange("(a p) f -> p a f", p=P), ob[:, 0:n, :])

            for vb in range(VT // OBAT):
                head_batch(vb * OBAT, OBAT)
            if VT % OBAT:
                head_batch((VT // OBAT) * OBAT, VT % OBAT)

    nc.compile()
    return nc


_NC_CACHE = None


def kernel(idx, tok_emb, pos_emb, ln1_g, ln1_b, qkv_w, proj_w,
           ln2_g, ln2_b, mlp_w1, mlp_b1, mlp_w2, mlp_b2,
           lnf_g, lnf_b, head_w, _trace=False):
    global _NC_CACHE
    import ml_dtypes
    BF = ml_dtypes.bfloat16
    idx = np.asarray(idx)
    f32 = lambda a: np.ascontiguousarray(np.asarray(a), dtype=np.float32)

    tok_emb, pos_emb = f32(tok_emb), f32(pos_emb)
    qkvwT = f32(qkv_w).transpose(0, 2, 1).astype(BF)     # [L, D, 3D]
    projwT = f32(proj_w).transpose(0, 2, 1).astype(BF)   # [L, D, D]
    w1T = f32(mlp_w1).transpose(0, 2, 1).astype(BF)      # [L, D, FF]
    w2T = f32(mlp_w2).transpose(0, 2, 1).astype(BF)      # [L, FF, D]
    headT = f32(head_w).T.astype(BF)                     # [D, V]

    # embedding on host
    x0 = tok_emb[idx] + pos_emb[0][None, :, :]           # [B, T, D]

    # packed small params [P, PKW]
    pkv = np.zeros((P, PKW), np.float32)
    ln1_g, ln1_b = f32(ln1_g), f32(ln1_b)
    ln2_g, ln2_b = f32(ln2_g), f32(ln2_b)
    b1v, b2v = f32(mlp_b1), f32(mlp_b2)
    for li in range(L):
        for k in range(CT):
            pkv[:, LAB_O + li * 2 * CT + 2 * k] = ln1_g[li, k * P:(k + 1) * P]
            pkv[:, LAB_O + li * 2 * CT + 2 * k + 1] = ln1_b[li, k * P:(k + 1) * P]
            pkv[:, LAB2_O + li * 2 * CT + 2 * k] = ln2_g[li, k * P:(k + 1) * P]
            pkv[:, LAB2_O + li * 2 * CT + 2 * k + 1] = ln2_b[li, k * P:(k + 1) * P]
        for k in range(FT):
            pkv[:, BB1_O + li * FT + k] = b1v[li, k * P:(k + 1) * P]
        for k in range(CT):
            pkv[:, BB2_O + li * CT + k] = b2v[li, k * P:(k + 1) * P]
    lnf_off = L * 2 * CT
    for k in range(CT):
        pkv[:, LAB_O + lnf_off + k] = f32(lnf_g)[k * P:(k + 1) * P]
        pkv[:, LAB2_O + lnf_off + k] = f32(lnf_b)[k * P:(k + 1) * P]

    # causal diagonal masks [4, 128, 512]
    masks = np.zeros((4, P, TL), np.float32)
    for j in range(4):
        for kl in range(P):
            masks[j, kl, j * P + kl:] = 1.0
    onesv = np.ones((P, 1), np.float32)
    ident = np.eye(P, dtype=np.float32)

    if _NC_CACHE is None:
        _NC_CACHE = build_program()
    nc = _NC_CACHE

    common = dict(qkvw=qkvwT, projw=projwT, w1=w1T, w2=w2T, headw=headT,
                  packed=pkv, masks=masks, onesd=onesv,
                  onesrd=np.ones((1, P), np.float32), identd=ident)
    in_maps = []
    for c in range(NCORES):
        b, s = c // 2, c % 2
        m = dict(common)
        m["x0T"] = np.ascontiguousarray(x0[b][s * TL:(s + 1) * TL, :].T)
        m["rbias"] = np.full((P, 1), 0.0 if s == 1 else -100.0, np.float32)
        in_maps.append(m)

    res = run_bass_kernel_spmd(nc, in_maps, list(range(NCORES)), trace=_trace)
    if getattr(res, "exec_time_ns", None):
        print(f"HW exec time: {res.exec_time_ns} ns")

    logits = np.empty((B, T, V), np.float32)
    for c in range(NCORES):
        b, s = c // 2, c % 2
        o = res.results[c]["out"]                        # [V, TL] bf16
        logits[b, s * TL:(s + 1) * TL, :] = np.asarray(o, dtype=np.float32).T
    return logits
